# revision 20
# baseline (speedup 1.0000x reference)
"""Causal multi-head attention (B=2, S=2048, E=1024, H=16, D=64) on 8 TRN2 cores.

Sharding: core c handles batch b = c//4 and head-group hg = c%4 (4 heads each).
Each core computes Q/K/V projections for its 4 heads, causal flash-style
attention, and a PARTIAL output projection against its 256-row slice of Wo.
Host sums the 4 partials per batch and adds bo.

Device-side layout tricks (all matmul operands bf16, f32 accumulation):
 - Host passes x pre-transposed (xT [E, S]) so Q^T/K^T/V come out of matmuls
   in exactly the layouts later stages need -> zero on-device transposes.
 - Scores are computed transposed (ST[t, s]) so softmax normalization uses a
   ones-column augmented V matmul for the denominators.
 - Heads are packed in pairs on partitions (0-63 / 64-127) so the K=64 score
   matmuls can run concurrently in separate PE row groups.
 - V is stored per (t-block, head-pair) as [V_even(64) | ones(1) | zeros(63)
   | V_odd(64)] so the even lhsT [128,65] puts oT_even at partitions 0-63
   (sums at 64) and the odd lhsT [128,128] puts oT_odd at partitions 64-127
   (sums at 0) -- partition-aligned for the later divides and Wo matmul.
 - Softmax denominators: raw sums are PE-broadcast to all partitions first,
   then ONE partition-parallel reciprocal_approx_fast (InstReciprocal on a
   single partition costs ~8 cycles/element and was the old bottleneck).
 - PSUM: scores get a dedicated 2-deep ring (2 banks each); kq/v/y/bcast
   scratch gets its own 2-deep ring of 1-bank tiles; AV accumulator 2 banks.
   (8 banks total; a shared ring used to stall the PE on pool rotation.)
"""

import ml_dtypes
import numpy as np

import concourse.bass as bass
import concourse.mybir as mybir
import concourse.tile as tile
from concourse import bacc
from concourse.bass_utils import run_bass_kernel_spmd

B, S, E, H, D = 2, 2048, 1024, 16, 64
P = 128
HPC = 4            # heads per core
HP = 2             # head pairs per core
EB = E // P        # 8 contraction blocks
TB = S // P        # 16 key/time blocks
CHW = 512          # query chunk width
CH = S // CHW      # 4 chunks
VW = 192           # per (t-block, head-pair) V columns: 64 + 1 + 63 + 64
NCORES = 8
SCALE = 1.0 / float(np.sqrt(D))
MASK_VAL = -1e30

F32 = mybir.dt.float32
F32R = mybir.dt.float32r
BF16 = mybir.dt.bfloat16

_CACHE = {}
LAST_RESULTS = None


def _emit_kernel(tc, xT, wq, wk, wv, bq, bk, bv, wo, mask, y):
    nc = tc.nc
    ADD = mybir.AluOpType.add
    MUL = mybir.AluOpType.mult
    EXP = mybir.ActivationFunctionType.Exp
    IDENT = mybir.ActivationFunctionType.Identity

    with (
        nc.allow_low_precision(reason="bf16 matmul pipeline"),
        tc.tile_pool(name="constp", bufs=1) as constp,
        tc.tile_pool(name="datap", bufs=1) as datap,
        tc.tile_pool(name="wpool", bufs=1) as wpool,
        tc.tile_pool(name="sexpp", bufs=8) as sexpp,
        tc.tile_pool(name="sumsp", bufs=2) as sumsp,
        tc.tile_pool(name="youtp", bufs=3) as youtp,
        tc.tile_pool(name="ps_sc", bufs=2, space="PSUM") as ps_sc,
        tc.tile_pool(name="ps_ms", bufs=2, space="PSUM") as ps_ms,
        tc.tile_pool(name="ps_ot", bufs=1, space="PSUM") as ps_ot,
    ):
        # ---- constants + weights ----------------------------------------
        mask_sb = constp.tile([P, P], BF16, name="mask_sb")
        ones_sb = constp.tile([P, 64], BF16, name="ones_sb")
        nc.vector.memset(ones_sb[:], 1.0)
        warm_sb = constp.tile([P, CHW], BF16, name="warm_sb")
        nc.vector.memset(warm_sb[:], 0.0)

        wq_sb = wpool.tile([P, EB, HPC * D], BF16, name="wq_sb")
        wk_sb = wpool.tile([P, EB, HPC * D], BF16, name="wk_sb")
        wv_sb = wpool.tile([P, EB, HPC * D], BF16, name="wv_sb")
        bq_sb = wpool.tile([P, HP], F32, name="bq_sb")
        bk_sb = wpool.tile([P, HP], F32, name="bk_sb")
        bv_sb = wpool.tile([P, HPC * D], F32, name="bv_sb")

        # ---- persistent data -------------------------------------------
        xT_sb = datap.tile([P, EB, S], BF16, name="xT_sb")
        qT_sb = datap.tile([P, HP, S], BF16, name="qT_sb")
        kT_sb = datap.tile([P, HP, S], BF16, name="kT_sb")
        v_sb = datap.tile([P, TB, HP, VW], BF16, name="v_sb")
        oT_sb = datap.tile([P, HP, S], BF16, name="oT_sb")
        wo_sb = datap.tile([P, HP, E], BF16, name="wo_sb")

        # DMA issue is SERIAL on the queue engine (~600ns per dma_start
        # regardless of size), so use few big DMAs ordered by first use:
        # wk + xT-chunk0 gate the first projection matmuls.
        xTr = xT.rearrange("(eb p) s -> p eb s", p=P)
        nc.sync.dma_start(wk_sb[:], wk.rearrange("(eb p) m -> p eb m", p=P))
        nc.sync.dma_start(xT_sb[:, 0:4, 0:CHW], xTr[:, 0:4, 0:CHW])
        nc.sync.dma_start(xT_sb[:, 4:8, 0:CHW], xTr[:, 4:8, 0:CHW])
        nc.sync.dma_start(wq_sb[:], wq.rearrange("(eb p) m -> p eb m", p=P))
        nc.sync.dma_start(mask_sb[:], mask[:, :])
        nc.sync.dma_start(bq_sb[:], bq[:, :])
        nc.sync.dma_start(bk_sb[:], bk[:, :])
        nc.sync.dma_start(bv_sb[:], bv[:, :])
        nc.sync.dma_start(wv_sb[:], wv.rearrange("(eb p) m -> p eb m", p=P))
        for c in range(1, CH):
            nc.sync.dma_start(
                xT_sb[:, :, c * CHW : (c + 1) * CHW],
                xTr[:, :, c * CHW : (c + 1) * CHW],
            )
            if c == 1:
                nc.sync.dma_start(wo_sb[:], wo.rearrange("(kb p) n -> p kb n", p=P))

        # V augmentation regions: shared ones column + zeros pad
        nc.vector.memset(v_sb[:, :, :, 64:65], 1.0)
        nc.vector.memset(v_sb[:, :, :, 65:128], 0.0)

        # PE warmup spin while input DMAs land (keeps HAM from idling cold)
        for w in range(12):
            wps = ps_ms.tile([P, CHW], F32, tag="ms", name="wps")
            nc.tensor.matmul(
                wps[:, :], warm_sb[:, 0:128], warm_sb[:], start=True, stop=True
            )

        # ---- deferred-work queue ----------------------------------------
        # Items are emitted between attention strips so the in-order PE
        # queue always has ready work ahead of instructions that might
        # wait on ACT/DVE round trips. FIFO order preserves RAW emission
        # order (norm before yproj; projections before their chunk).
        pending = []

        def flush_one():
            if pending:
                pending.pop(0)[2]()

        def make_kqproj(c, hp):
            cs = slice(c * CHW, (c + 1) * CHW)
            hs = slice(hp * P, (hp + 1) * P)

            def emit():
                kps = ps_ms.tile([P, CHW], F32, tag="ms", name="kps")
                for e in range(EB):
                    nc.tensor.matmul(
                        kps[:, :],
                        wk_sb[:, e, hs],
                        xT_sb[:, e, cs],
                        start=(e == 0),
                        stop=(e == EB - 1),
                    )
                nc.scalar.activation(
                    kT_sb[:, hp, cs], kps[:, :], IDENT, bias=bk_sb[:, hp : hp + 1]
                )
                qps = ps_ms.tile([P, CHW], F32, tag="ms", name="qps")
                for e in range(EB):
                    nc.tensor.matmul(
                        qps[:, :],
                        wq_sb[:, e, hs],
                        xT_sb[:, e, cs],
                        start=(e == 0),
                        stop=(e == EB - 1),
                    )
                nc.scalar.activation(
                    qT_sb[:, hp, cs], qps[:, :], IDENT, bias=bq_sb[:, hp : hp + 1]
                )

            return emit

        def make_vproj(tb):
            def emit():
                vps = ps_ms.tile([P, CHW], F32, tag="ms", name="vps")
                for e in range(EB):
                    nc.tensor.matmul(
                        vps[:, 0 : HPC * D],
                        xT_sb[:, e, tb * P : (tb + 1) * P],
                        wv_sb[:, e, :],
                        start=(e == 0),
                        stop=(e == EB - 1),
                    )
                pv = vps[:, 0 : HPC * D].rearrange(
                    "p (hp j d) -> p hp j d", hp=HP, j=2
                )
                bvv = bv_sb.rearrange("p (hp j d) -> p hp j d", hp=HP, j=2)
                nc.vector.tensor_tensor(
                    v_sb[:, tb, :, 0:64], pv[:, :, 0, :], bvv[:, :, 0, :], ADD
                )
                nc.vector.tensor_tensor(
                    v_sb[:, tb, :, 128:192], pv[:, :, 1, :], bvv[:, :, 1, :], ADD
                )

            return emit

        def emit_bcast_recip(sums_bf):
            """Broadcast the two 1-partition sums rows to [128, CHW] via K=1
            matmuls, then one partition-parallel fast reciprocal."""
            bc_ps = ps_ms.tile([P, CHW], F32, tag="ms", name="bc_ps")
            nc.tensor.matmul(
                bc_ps[0:64, :],
                ones_sb[64:65, :],
                sums_bf[64:65, :],
                start=True,
                stop=True,
                skip_group_check=True,
            )
            nc.tensor.matmul(
                bc_ps[64:128, :],
                ones_sb[0:1, :],
                sums_bf[0:1, :],
                start=True,
                stop=True,
                tile_position=(0, 64),
                skip_group_check=True,
            )
            rec = sumsp.tile([P, CHW], F32, tag="rec", name="rec")
            nc.vector.reciprocal_approx_fast(rec[:, :], bc_ps[:, :])
            return rec

        def make_norm(hp, cs, ot_u, sums_bf):
            def emit():
                rec = emit_bcast_recip(sums_bf)
                nc.vector.tensor_tensor(
                    oT_sb[0:64, hp, cs], ot_u[0:64, 0, :], rec[0:64, :], MUL
                )
                nc.vector.tensor_tensor(
                    oT_sb[64:128, hp, cs], ot_u[64:128, 1, :], rec[64:128, :], MUL
                )

            return emit

        # per-chunk y staging: 4 s-blocks land in one [P, 4, E] tile, then a
        # single DMA writes all 512 rows (DMA issue is ~600ns each, serial)
        ychunk = {}
        yr = y.rearrange("(sb p) n -> p sb n", p=P)

        def make_yproj(c, sbi):
            def emit():
                if sbi == 0:
                    ychunk[c] = youtp.tile([P, 4, E], F32, tag="y", bufs=2, name="y_t")
                y_t = ychunk[c]
                sb = 4 * c + sbi
                for nt in range(2):
                    yps = ps_ms.tile([P, CHW], F32, tag="ms", name="yps")
                    for kb in range(HP):
                        nc.tensor.matmul(
                            yps[:, :],
                            oT_sb[:, kb, sb * P : (sb + 1) * P],
                            wo_sb[:, kb, nt * CHW : (nt + 1) * CHW],
                            start=(kb == 0),
                            stop=(kb == HP - 1),
                        )
                    nc.vector.tensor_copy(
                        y_t[:, sbi, nt * CHW : (nt + 1) * CHW], yps[:, :]
                    )
                if sbi == 3:
                    nc.sync.dma_start(yr[:, 4 * c : 4 * c + 4, :], y_t[:])

            return emit

        def emit_tail(c, ot_u, sums_bf):
            """Last (c, hp): run norm + output projection immediately,
            split per s-block so the drain pipelines across PE/ACT/DVE
            instead of serializing full-chunk stages."""
            hp = HP - 1
            rec = emit_bcast_recip(sums_bf)
            for sbi in range(4):
                sb = 4 * c + sbi
                ws = slice(sbi * P, (sbi + 1) * P)
                gs = slice(c * CHW + sbi * P, c * CHW + (sbi + 1) * P)
                nc.vector.tensor_tensor(
                    oT_sb[0:64, hp, gs], ot_u[0:64, 0, ws], rec[0:64, ws], MUL
                )
                nc.vector.tensor_tensor(
                    oT_sb[64:128, hp, gs], ot_u[64:128, 1, ws], rec[64:128, ws], MUL
                )
                y_t = youtp.tile([P, E], F32, tag="yt", bufs=2, name="y_tail")
                for nt in range(2):
                    # scores are finished: reuse the freed score PSUM ring so
                    # the yproj matmuls don't stall on evacuation round-trips
                    yps = ps_sc.tile([P, 2, CHW], F32, tag="sc", name="yps_t")
                    for kb in range(HP):
                        nc.tensor.matmul(
                            yps[:, 0, :],
                            oT_sb[:, kb, sb * P : (sb + 1) * P],
                            wo_sb[:, kb, nt * CHW : (nt + 1) * CHW],
                            start=(kb == 0),
                            stop=(kb == HP - 1),
                        )
                    # exps are done by now, so the ACT engine is free: use it
                    # for the evacuation.
                    nc.scalar.copy(y_t[:, nt * CHW : (nt + 1) * CHW], yps[:, 0, :])
                nc.sync.dma_start(y[sb * P : (sb + 1) * P, :], y_t[:])

        # chunk-0 projections run directly; later chunks stream in as filler
        for hp in range(HP):
            make_kqproj(0, hp)()

        for c in range(CH):
            cs = slice(c * CHW, (c + 1) * CHW)
            if c > 0:
                for hp in range(HP):
                    make_kqproj(c, hp)()

            for hp in range(HP):
                ot_ps = ps_ot.tile([P, 2, CHW], F32, tag="ot", name="ot_ps")
                nstrips = 4 * c + 4
                kos = [max(0, tj - 4 * c) * P for tj in range(nstrips)]
                exs = [None] * nstrips

                def emit_scores(i, c=c, hp=hp, kos=kos, exs=exs):
                    ko = kos[i]
                    sc_ps = ps_sc.tile([P, 2, CHW], F32, tag="sc", name="sc_ps")
                    for j in range(2):
                        off = j * 64
                        nc.tensor.matmul(
                            sc_ps[:, j, ko:CHW],
                            kT_sb[off : off + 64, hp, i * P : (i + 1) * P],
                            qT_sb[off : off + 64, hp, c * CHW + ko : (c + 1) * CHW],
                            start=True,
                            stop=True,
                        )
                    ex = sexpp.tile([P, 2, CHW], BF16, tag="ex", name="ex")
                    nc.scalar.activation(
                        ex[:, :, ko:CHW], sc_ps[:, :, ko:CHW], EXP, scale=SCALE
                    )
                    if i - 4 * c >= 0:
                        nc.vector.tensor_tensor(
                            ex[:, :, ko : ko + P],
                            ex[:, :, ko : ko + P],
                            mask_sb[:, None, :].broadcast_to((P, 2, P)),
                            MUL,
                        )
                    exs[i] = ex

                def emit_av(i, c=c, hp=hp, kos=kos, exs=exs, ot_ps=ot_ps,
                            nstrips=nstrips):
                    ko = kos[i]
                    last = i == nstrips - 1
                    nc.tensor.matmul(
                        ot_ps[0:65, 0, ko:CHW],
                        v_sb[:, i, hp, 0:65],
                        exs[i][:, 0, ko:CHW],
                        start=(i == 0),
                        stop=last,
                        skip_group_check=True,
                    )
                    nc.tensor.matmul(
                        ot_ps[:, 1, ko:CHW],
                        v_sb[:, i, hp, 64:192],
                        exs[i][:, 1, ko:CHW],
                        start=(i == 0),
                        stop=last,
                        skip_group_check=True,
                    )

                # AV trails scores by 3 strips so the PE has queued score
                # work to cover the previous accumulator's DVE evacuation.
                lag = min(3, nstrips - 1)
                for i in range(nstrips):
                    emit_scores(i)
                    if hp == 0 and i < 4:
                        make_vproj(4 * c + i)()
                    if i >= 3 and i % 2 == 1:
                        flush_one()
                    if i >= lag:
                        emit_av(i - lag)
                for i in range(nstrips - lag, nstrips):
                    emit_av(i)

                ot_u = sumsp.tile([P, 2, CHW], F32, tag="otu", bufs=2, name="ot_u")
                sums_bf = sumsp.tile([P, CHW], BF16, tag="sums", name="sums_bf")
                if c == CH - 1 and hp == HP - 1:
                    # tail: the bf16 sums casts gate the broadcast+reciprocal
                    # chain, so issue them first (straight from PSUM) and put
                    # one evacuation on the now-idle ACT engine.
                    nc.vector.tensor_copy(sums_bf[64:65, :], ot_ps[64:65, 0, :])
                    nc.vector.tensor_copy(sums_bf[0:1, :], ot_ps[0:1, 1, :])
                    nc.vector.tensor_copy(ot_u[0:65, 0, :], ot_ps[0:65, 0, :])
                    nc.scalar.copy(ot_u[:, 1, :], ot_ps[:, 1, :])
                    emit_tail(c, ot_u, sums_bf)
                else:
                    # evacuate the accumulator first (frees the single psum
                    # slot for the next head-pair), then the bf16 sums rows
                    # for the later PE broadcast
                    nc.vector.tensor_copy(ot_u[0:65, 0, :], ot_ps[0:65, 0, :])
                    nc.vector.tensor_copy(ot_u[:, 1, :], ot_ps[:, 1, :])
                    nc.vector.tensor_copy(sums_bf[64:65, :], ot_u[64:65, 0, :])
                    nc.vector.tensor_copy(sums_bf[0:1, :], ot_u[0:1, 1, :])
                    pending.append(("norm", c, make_norm(hp, cs, ot_u, sums_bf)))
            if c < CH - 1:
                for sbi in range(4):
                    pending.append(("y", c, make_yproj(c, sbi)))
        while pending:
            pending.pop(0)[2]()


def build():
    if "nc" in _CACHE:
        return _CACHE["nc"]
    nc = bacc.Bacc("TRN2", target_bir_lowering=False, debug=False, enable_asserts=False)
    xT = nc.dram_tensor("xT", (E, S), BF16, kind="ExternalInput").ap()
    wq = nc.dram_tensor("wq", (E, HPC * D), BF16, kind="ExternalInput").ap()
    wk = nc.dram_tensor("wk", (E, HPC * D), BF16, kind="ExternalInput").ap()
    wv = nc.dram_tensor("wv", (E, HPC * D), BF16, kind="ExternalInput").ap()
    bq = nc.dram_tensor("bq", (P, HP), F32, kind="ExternalInput").ap()
    bk = nc.dram_tensor("bk", (P, HP), F32, kind="ExternalInput").ap()
    bv = nc.dram_tensor("bv", (P, HPC * D), F32, kind="ExternalInput").ap()
    wo = nc.dram_tensor("wo", (HPC * D, E), BF16, kind="ExternalInput").ap()
    mask = nc.dram_tensor("mask", (P, P), BF16, kind="ExternalInput").ap()
    y = nc.dram_tensor("y", (S, E), F32, kind="ExternalOutput").ap()
    with tile.TileContext(nc) as tc:
        _emit_kernel(tc, xT, wq, wk, wv, bq, bk, bv, wo, mask, y)
    nc.compile()
    _CACHE["nc"] = nc
    return nc


def make_in_maps(x, Wq, bq, Wk, bk, Wv, bv, Wo):
    bf16 = ml_dtypes.bfloat16
    xTs = [np.ascontiguousarray(x[b].T.astype(bf16)) for b in range(B)]
    ti = np.arange(P)[:, None]
    si = np.arange(P)[None, :]
    mask = np.where(si >= ti, 1.0, 0.0).astype(bf16)
    in_maps = []
    for core in range(NCORES):
        b, hg = core // HPC, core % HPC
        hs = slice(hg * HPC, (hg + 1) * HPC)

        def wmat(W):
            return np.ascontiguousarray(
                W[hs].transpose(1, 0, 2).reshape(E, HPC * D).astype(bf16)
            )

        def bpair(bias):
            flat = bias[hs].reshape(HPC * D).astype(np.float32)
            return np.ascontiguousarray(flat.reshape(HP, P).T)

        bv_flat = bv[hs].reshape(HPC * D).astype(np.float32)
        in_maps.append(
            {
                "xT": xTs[b],
                "wq": wmat(Wq),
                "wk": wmat(Wk),
                "wv": wmat(Wv),
                "bq": bpair(bq),
                "bk": bpair(bk),
                "bv": np.ascontiguousarray(np.tile(bv_flat, (P, 1))),
                "wo": np.ascontiguousarray(
                    Wo[hg * HPC * D : (hg + 1) * HPC * D].astype(bf16)
                ),
                "mask": mask,
            }
        )
    return in_maps


def kernel(**inputs):
    global LAST_RESULTS
    x = np.asarray(inputs["x"], dtype=np.float32)
    Wq = np.asarray(inputs["Wq"], dtype=np.float32)
    bq = np.asarray(inputs["bq"], dtype=np.float32)
    Wk = np.asarray(inputs["Wk"], dtype=np.float32)
    bk = np.asarray(inputs["bk"], dtype=np.float32)
    Wv = np.asarray(inputs["Wv"], dtype=np.float32)
    bv = np.asarray(inputs["bv"], dtype=np.float32)
    Wo = np.asarray(inputs["Wo"], dtype=np.float32)
    bo = np.asarray(inputs["bo"], dtype=np.float32)

    nc = build()
    in_maps = make_in_maps(x, Wq, bq, Wk, bk, Wv, bv, Wo)
    res = run_bass_kernel_spmd(nc, in_maps, core_ids=list(range(NCORES)))
    LAST_RESULTS = res

    y = np.zeros((B, S, E), dtype=np.float32)
    for core in range(NCORES):
        y[core // HPC] += res.results[core]["y"]
    y += bo[None, None, :]
    return y


# revision 42
# speedup vs baseline: 1.0718x; 1.0718x over previous
"""Causal multi-head attention (B=2, S=2048, E=1024, H=16, D=64) on 8 TRN2 cores.

Sharding: core c handles batch b = c//4 and head-group hg = c%4 (4 heads each).
Each core computes Q/K/V projections for its 4 heads, causal flash-style
attention, and a PARTIAL output projection against its 256-row slice of Wo.
Host sums the 4 partials per batch and adds bo.

Device-side layout tricks (all matmul operands bf16, f32 accumulation):
 - Host passes x pre-transposed (xT [E, S]) so Q^T/K^T/V come out of matmuls
   in exactly the layouts later stages need -> zero on-device transposes.
 - Scores are computed transposed (ST[t, s]) so softmax normalization uses a
   ones-column augmented V matmul for the denominators.
 - Heads are packed in pairs on partitions (0-63 / 64-127) so the K=64 score
   matmuls can run concurrently in separate PE row groups.
 - V is stored per (t-block, head-pair) as [V_even(64) | ones(1) | zeros(63)
   | V_odd(64)] so the even lhsT [128,65] puts oT_even at partitions 0-63
   (sums at 64) and the odd lhsT [128,128] puts oT_odd at partitions 64-127
   (sums at 0) -- partition-aligned for the later divides and Wo matmul.
 - Softmax denominators: raw sums are PE-broadcast to all partitions first,
   then ONE partition-parallel reciprocal_approx_fast (InstReciprocal on a
   single partition costs ~8 cycles/element and was the old bottleneck).
 - PSUM: scores get a dedicated 2-deep ring (2 banks each); kq/v/y/bcast
   scratch gets its own 2-deep ring of 1-bank tiles; AV accumulator 2 banks.
   (8 banks total; a shared ring used to stall the PE on pool rotation.)
"""

import ml_dtypes
import numpy as np

import concourse.bass as bass
import concourse.mybir as mybir
import concourse.tile as tile
from concourse import bacc
from concourse.bass_utils import run_bass_kernel_spmd

B, S, E, H, D = 2, 2048, 1024, 16, 64
P = 128
HPC = 4            # heads per core
HP = 2             # head pairs per core
EB = E // P        # 8 contraction blocks
TB = S // P        # 16 key/time blocks
CHW = 512          # query chunk width
CH = S // CHW      # 4 chunks
VW = 192           # per (t-block, head-pair) V columns: 64 + 1 + 63 + 64
NCORES = 8
SCALE = 1.0 / float(np.sqrt(D))
MASK_VAL = -1e30

F32 = mybir.dt.float32
F32R = mybir.dt.float32r
BF16 = mybir.dt.bfloat16
F8 = mybir.dt.float8e4
DR = mybir.MatmulPerfMode.DoubleRow
WSCALE = 32.0       # host-side weight scale: keeps fp8 weights out of denormals

_CACHE = {}
LAST_RESULTS = None


def _emit_kernel(tc, xT, xb, wq, wk, wv, bq, bk, bv, wo, mask, y):
    nc = tc.nc
    ADD = mybir.AluOpType.add
    MUL = mybir.AluOpType.mult
    EXP = mybir.ActivationFunctionType.Exp
    IDENT = mybir.ActivationFunctionType.Identity
    # Q/K carry a WSCALE^2 factor (fp8 weights are pre-scaled); fold the
    # correction into the softmax exp scale.
    ESCALE = SCALE / (WSCALE * WSCALE)

    with (
        nc.allow_low_precision(reason="bf16 matmul pipeline"),
        tc.tile_pool(name="constp", bufs=1) as constp,
        tc.tile_pool(name="datap", bufs=1) as datap,
        tc.tile_pool(name="wpool", bufs=1) as wpool,
        tc.tile_pool(name="sexpp", bufs=8) as sexpp,
        tc.tile_pool(name="sumsp", bufs=2) as sumsp,
        tc.tile_pool(name="youtp", bufs=3) as youtp,
        tc.tile_pool(name="ps_sc", bufs=2, space="PSUM") as ps_sc,
        tc.tile_pool(name="ps_ms", bufs=2, space="PSUM") as ps_ms,
        tc.tile_pool(name="ps_ot", bufs=1, space="PSUM") as ps_ot,
    ):
        # ---- constants + weights ----------------------------------------
        mask_sb = constp.tile([P, P], BF16, name="mask_sb")
        ones_sb = constp.tile([P, 64], BF16, name="ones_sb")
        nc.vector.memset(ones_sb[:], 1.0)
        warm_sb = constp.tile([P, CHW], BF16, name="warm_sb")
        nc.vector.memset(warm_sb[:], 0.0)

        wq_sb = wpool.tile([P, EB, HPC * D], F8, name="wq_sb")
        wk_sb = wpool.tile([P, EB, HPC * D], F8, name="wk_sb")
        wv_sb = wpool.tile([P, EB, HPC * D], BF16, name="wv_sb")
        bq_sb = wpool.tile([P, HP], F32, name="bq_sb")
        bk_sb = wpool.tile([P, HP], F32, name="bk_sb")
        bv_sb = wpool.tile([P, HPC * D], F32, name="bv_sb")

        # ---- persistent data -------------------------------------------
        # x is staged twice: fp8 for the DoubleRow Q/K projections (errors
        # there only perturb softmax weights) and bf16 for the V projection
        # (V quantization error passes straight through to the output).
        xT_sb = datap.tile([P, EB, S], F8, name="xT_sb")
        xb_sb = datap.tile([P, EB, S], BF16, name="xb_sb")
        qT_sb = datap.tile([P, HP, S], BF16, name="qT_sb")
        kT_sb = datap.tile([P, HP, S], BF16, name="kT_sb")
        v_sb = datap.tile([P, TB, HP, VW], BF16, name="v_sb")
        oT_sb = datap.tile([P, HP, S], BF16, name="oT_sb")
        wo_sb = datap.tile([P, HP, E], BF16, name="wo_sb")

        # DMA issue is SERIAL on the queue engine (~600ns per dma_start
        # regardless of size), so use few big DMAs ordered by first use:
        # wk + xT-chunk0 gate the first projection matmuls.
        xTr = xT.rearrange("(eb p) s -> p eb s", p=P)
        xbr = xb.rearrange("(eb p) s -> p eb s", p=P)
        nc.sync.dma_start(wk_sb[:], wk.rearrange("(eb p) m -> p eb m", p=P))
        nc.sync.dma_start(xT_sb[:, 0:4, 0:CHW], xTr[:, 0:4, 0:CHW])
        nc.sync.dma_start(xT_sb[:, 4:8, 0:CHW], xTr[:, 4:8, 0:CHW])
        nc.sync.dma_start(wq_sb[:], wq.rearrange("(eb p) m -> p eb m", p=P))
        nc.sync.dma_start(xb_sb[:, :, 0:CHW], xbr[:, :, 0:CHW])
        nc.sync.dma_start(mask_sb[:], mask[:, :])
        nc.sync.dma_start(bq_sb[:], bq[:, :])
        nc.sync.dma_start(bk_sb[:], bk[:, :])
        nc.sync.dma_start(bv_sb[:], bv[:, :])
        nc.sync.dma_start(wv_sb[:], wv.rearrange("(eb p) m -> p eb m", p=P))
        nc.sync.dma_start(xT_sb[:, :, CHW:S], xTr[:, :, CHW:S])
        nc.sync.dma_start(xb_sb[:, :, CHW:S], xbr[:, :, CHW:S])
        nc.sync.dma_start(wo_sb[:], wo.rearrange("(kb p) n -> p kb n", p=P))

        # V augmentation regions: shared ones column + zeros pad
        nc.vector.memset(v_sb[:, :, :, 64:65], 1.0)
        nc.vector.memset(v_sb[:, :, :, 65:128], 0.0)

        # PE warmup spin while input DMAs land (keeps HAM from idling cold)
        for w in range(12):
            wps = ps_ms.tile([P, CHW], F32, tag="ms", name="wps")
            nc.tensor.matmul(
                wps[:, :], warm_sb[:, 0:128], warm_sb[:], start=True, stop=True
            )

        # ---- deferred-work queue ----------------------------------------
        # Items are emitted between attention strips so the in-order PE
        # queue always has ready work ahead of instructions that might
        # wait on ACT/DVE round trips. FIFO order preserves RAW emission
        # order (norm before yproj; projections before their chunk).
        pending = []

        def flush_one():
            if pending:
                pending.pop(0)[2]()

        def make_kqproj(c, hp):
            cs = slice(c * CHW, (c + 1) * CHW)
            hs = slice(hp * P, (hp + 1) * P)

            def emit():
                # fp8 DoubleRow: two e-blocks (256 contraction rows) per matmul
                kps = ps_ms.tile([P, CHW], F32, tag="ms", name="kps")
                for e in range(EB // 2):
                    nc.tensor.matmul(
                        kps[:, :],
                        wk_sb[:, 2 * e : 2 * e + 2, hs],
                        xT_sb[:, 2 * e : 2 * e + 2, cs],
                        start=(e == 0),
                        stop=(e == EB // 2 - 1),
                        perf_mode=DR,
                    )
                nc.scalar.activation(
                    kT_sb[:, hp, cs], kps[:, :], IDENT, bias=bk_sb[:, hp : hp + 1]
                )
                qps = ps_ms.tile([P, CHW], F32, tag="ms", name="qps")
                for e in range(EB // 2):
                    nc.tensor.matmul(
                        qps[:, :],
                        wq_sb[:, 2 * e : 2 * e + 2, hs],
                        xT_sb[:, 2 * e : 2 * e + 2, cs],
                        start=(e == 0),
                        stop=(e == EB // 2 - 1),
                        perf_mode=DR,
                    )
                nc.scalar.activation(
                    qT_sb[:, hp, cs], qps[:, :], IDENT, bias=bq_sb[:, hp : hp + 1]
                )

            return emit

        def make_vproj(tb):
            def emit():
                vps = ps_ms.tile([P, CHW], F32, tag="ms", name="vps")
                for e in range(EB):
                    nc.tensor.matmul(
                        vps[:, 0 : HPC * D],
                        xb_sb[:, e, tb * P : (tb + 1) * P],
                        wv_sb[:, e, :],
                        start=(e == 0),
                        stop=(e == EB - 1),
                    )
                pv = vps[:, 0 : HPC * D].rearrange(
                    "p (hp j d) -> p hp j d", hp=HP, j=2
                )
                bvv = bv_sb.rearrange("p (hp j d) -> p hp j d", hp=HP, j=2)
                nc.vector.tensor_tensor(
                    v_sb[:, tb, :, 0:64], pv[:, :, 0, :], bvv[:, :, 0, :], ADD
                )
                nc.vector.tensor_tensor(
                    v_sb[:, tb, :, 128:192], pv[:, :, 1, :], bvv[:, :, 1, :], ADD
                )

            return emit

        def emit_bcast_recip(sums_bf):
            """Broadcast the two 1-partition sums rows to [128, CHW] via K=1
            matmuls, then one partition-parallel fast reciprocal."""
            bc_ps = ps_ms.tile([P, CHW], F32, tag="ms", name="bc_ps")
            nc.tensor.matmul(
                bc_ps[0:64, :],
                ones_sb[64:65, :],
                sums_bf[64:65, :],
                start=True,
                stop=True,
                skip_group_check=True,
            )
            nc.tensor.matmul(
                bc_ps[64:128, :],
                ones_sb[0:1, :],
                sums_bf[0:1, :],
                start=True,
                stop=True,
                tile_position=(0, 64),
                skip_group_check=True,
            )
            rec = sumsp.tile([P, CHW], F32, tag="rec", name="rec")
            nc.vector.reciprocal_approx_fast(rec[:, :], bc_ps[:, :])
            return rec

        def make_norm(hp, cs, ot_u, sums_bf):
            def emit():
                rec = emit_bcast_recip(sums_bf)
                nc.vector.tensor_tensor(
                    oT_sb[0:64, hp, cs], ot_u[0:64, 0, :], rec[0:64, :], MUL
                )
                nc.vector.tensor_tensor(
                    oT_sb[64:128, hp, cs], ot_u[64:128, 1, :], rec[64:128, :], MUL
                )

            return emit

        # per-chunk y staging: 4 s-blocks land in one [P, 4, E] tile, then a
        # single DMA writes all 512 rows (DMA issue is ~600ns each, serial)
        ychunk = {}
        yr = y.rearrange("(sb p) n -> p sb n", p=P)

        def make_yproj(c, sbi):
            def emit():
                if sbi == 0:
                    ychunk[c] = youtp.tile([P, 4, E], BF16, tag="y", bufs=2, name="y_t")
                y_t = ychunk[c]
                sb = 4 * c + sbi
                for nt in range(2):
                    yps = ps_ms.tile([P, CHW], F32, tag="ms", name="yps")
                    for kb in range(HP):
                        nc.tensor.matmul(
                            yps[:, :],
                            oT_sb[:, kb, sb * P : (sb + 1) * P],
                            wo_sb[:, kb, nt * CHW : (nt + 1) * CHW],
                            start=(kb == 0),
                            stop=(kb == HP - 1),
                        )
                    nc.vector.tensor_copy(
                        y_t[:, sbi, nt * CHW : (nt + 1) * CHW], yps[:, :]
                    )
                if sbi == 3:
                    nc.sync.dma_start(yr[:, 4 * c : 4 * c + 4, :], y_t[:])

            return emit

        def emit_tail(c, ot_u, sums_bf):
            """Last (c, hp): run norm + output projection immediately,
            split per s-block so the drain pipelines across PE/ACT/DVE
            instead of serializing full-chunk stages."""
            hp = HP - 1
            rec = emit_bcast_recip(sums_bf)
            for sbi in range(4):
                sb = 4 * c + sbi
                ws = slice(sbi * P, (sbi + 1) * P)
                gs = slice(c * CHW + sbi * P, c * CHW + (sbi + 1) * P)
                nc.vector.tensor_tensor(
                    oT_sb[0:64, hp, gs], ot_u[0:64, 0, ws], rec[0:64, ws], MUL
                )
                nc.vector.tensor_tensor(
                    oT_sb[64:128, hp, gs], ot_u[64:128, 1, ws], rec[64:128, ws], MUL
                )
                y_t = youtp.tile([P, E], BF16, tag="yt", bufs=2, name="y_tail")
                for nt in range(2):
                    # scores are finished: reuse the freed score PSUM ring so
                    # the yproj matmuls don't stall on evacuation round-trips
                    yps = ps_sc.tile([P, 2, CHW], F32, tag="sc", name="yps_t")
                    for kb in range(HP):
                        nc.tensor.matmul(
                            yps[:, 0, :],
                            oT_sb[:, kb, sb * P : (sb + 1) * P],
                            wo_sb[:, kb, nt * CHW : (nt + 1) * CHW],
                            start=(kb == 0),
                            stop=(kb == HP - 1),
                        )
                    # exps are done by now, so the ACT engine is free: use it
                    # for the evacuation.
                    nc.scalar.copy(y_t[:, nt * CHW : (nt + 1) * CHW], yps[:, 0, :])
                nc.sync.dma_start(y[sb * P : (sb + 1) * P, :], y_t[:])

        # chunk-0 projections run directly; later chunks stream in as filler
        for hp in range(HP):
            make_kqproj(0, hp)()

        for c in range(CH):
            cs = slice(c * CHW, (c + 1) * CHW)
            if c > 0:
                for hp in range(HP):
                    make_kqproj(c, hp)()

            for hp in range(HP):
                ot_ps = ps_ot.tile([P, 2, CHW], F32, tag="ot", name="ot_ps")
                nstrips = 4 * c + 4
                kos = [max(0, tj - 4 * c) * P for tj in range(nstrips)]
                exs = [None] * nstrips

                def emit_scores(i, c=c, hp=hp, kos=kos, exs=exs):
                    ko = kos[i]
                    sc_ps = ps_sc.tile([P, 2, CHW], F32, tag="sc", name="sc_ps")
                    for j in range(2):
                        off = j * 64
                        nc.tensor.matmul(
                            sc_ps[:, j, ko:CHW],
                            kT_sb[off : off + 64, hp, i * P : (i + 1) * P],
                            qT_sb[off : off + 64, hp, c * CHW + ko : (c + 1) * CHW],
                            start=True,
                            stop=True,
                        )
                    ex = sexpp.tile([P, 2, CHW], BF16, tag="ex", name="ex")
                    nc.scalar.activation(
                        ex[:, :, ko:CHW], sc_ps[:, :, ko:CHW], EXP, scale=ESCALE
                    )
                    if i - 4 * c >= 0:
                        nc.vector.tensor_tensor(
                            ex[:, :, ko : ko + P],
                            ex[:, :, ko : ko + P],
                            mask_sb[:, None, :].broadcast_to((P, 2, P)),
                            MUL,
                        )
                    exs[i] = ex

                def emit_av(i, c=c, hp=hp, kos=kos, exs=exs, ot_ps=ot_ps,
                            nstrips=nstrips):
                    ko = kos[i]
                    last = i == nstrips - 1
                    nc.tensor.matmul(
                        ot_ps[0:65, 0, ko:CHW],
                        v_sb[:, i, hp, 0:65],
                        exs[i][:, 0, ko:CHW],
                        start=(i == 0),
                        stop=last,
                        skip_group_check=True,
                    )
                    nc.tensor.matmul(
                        ot_ps[:, 1, ko:CHW],
                        v_sb[:, i, hp, 64:192],
                        exs[i][:, 1, ko:CHW],
                        start=(i == 0),
                        stop=last,
                        skip_group_check=True,
                    )

                # AV trails scores by 3 strips so the PE has queued score
                # work to cover the previous accumulator's DVE evacuation.
                lag = min(3, nstrips - 1)
                for i in range(nstrips):
                    emit_scores(i)
                    if hp == 0 and i < 4:
                        make_vproj(4 * c + i)()
                    if i >= 3 and i % 2 == 1:
                        flush_one()
                    if i >= lag:
                        emit_av(i - lag)
                for i in range(nstrips - lag, nstrips):
                    emit_av(i)

                ot_u = sumsp.tile([P, 2, CHW], F32, tag="otu", bufs=2, name="ot_u")
                sums_bf = sumsp.tile([P, CHW], BF16, tag="sums", name="sums_bf")
                if c == CH - 1 and hp == HP - 1:
                    # tail: the bf16 sums casts gate the broadcast+reciprocal
                    # chain, so issue them first (straight from PSUM) and put
                    # one evacuation on the now-idle ACT engine.
                    nc.vector.tensor_copy(sums_bf[64:65, :], ot_ps[64:65, 0, :])
                    nc.vector.tensor_copy(sums_bf[0:1, :], ot_ps[0:1, 1, :])
                    nc.vector.tensor_copy(ot_u[0:65, 0, :], ot_ps[0:65, 0, :])
                    nc.scalar.copy(ot_u[:, 1, :], ot_ps[:, 1, :])
                    emit_tail(c, ot_u, sums_bf)
                else:
                    # evacuate the accumulator first (frees the single psum
                    # slot for the next head-pair), then the bf16 sums rows
                    # for the later PE broadcast
                    nc.vector.tensor_copy(ot_u[0:65, 0, :], ot_ps[0:65, 0, :])
                    nc.vector.tensor_copy(ot_u[:, 1, :], ot_ps[:, 1, :])
                    nc.vector.tensor_copy(sums_bf[64:65, :], ot_u[64:65, 0, :])
                    nc.vector.tensor_copy(sums_bf[0:1, :], ot_u[0:1, 1, :])
                    pending.append(("norm", c, make_norm(hp, cs, ot_u, sums_bf)))
            if c < CH - 1:
                for sbi in range(4):
                    pending.append(("y", c, make_yproj(c, sbi)))
        while pending:
            pending.pop(0)[2]()


def build():
    if "nc" in _CACHE:
        return _CACHE["nc"]
    nc = bacc.Bacc("TRN2", target_bir_lowering=False, debug=False, enable_asserts=False)
    xT = nc.dram_tensor("xT", (E, S), F8, kind="ExternalInput").ap()
    xb = nc.dram_tensor("xb", (E, S), BF16, kind="ExternalInput").ap()
    wq = nc.dram_tensor("wq", (E, HPC * D), F8, kind="ExternalInput").ap()
    wk = nc.dram_tensor("wk", (E, HPC * D), F8, kind="ExternalInput").ap()
    wv = nc.dram_tensor("wv", (E, HPC * D), BF16, kind="ExternalInput").ap()
    bq = nc.dram_tensor("bq", (P, HP), F32, kind="ExternalInput").ap()
    bk = nc.dram_tensor("bk", (P, HP), F32, kind="ExternalInput").ap()
    bv = nc.dram_tensor("bv", (P, HPC * D), F32, kind="ExternalInput").ap()
    wo = nc.dram_tensor("wo", (HPC * D, E), BF16, kind="ExternalInput").ap()
    mask = nc.dram_tensor("mask", (P, P), BF16, kind="ExternalInput").ap()
    y = nc.dram_tensor("y", (S, E), BF16, kind="ExternalOutput").ap()
    with tile.TileContext(nc) as tc:
        _emit_kernel(tc, xT, xb, wq, wk, wv, bq, bk, bv, wo, mask, y)
    nc.compile()
    _CACHE["nc"] = nc
    return nc


def make_in_maps(x, Wq, bq, Wk, bk, Wv, bv, Wo):
    bf16 = ml_dtypes.bfloat16
    fp8 = ml_dtypes.float8_e4m3fn
    xTs = [np.ascontiguousarray(x[b].T.astype(fp8)) for b in range(B)]
    xbs = [np.ascontiguousarray(x[b].T.astype(bf16)) for b in range(B)]
    ti = np.arange(P)[:, None]
    si = np.arange(P)[None, :]
    mask = np.where(si >= ti, 1.0, 0.0).astype(bf16)
    in_maps = []
    for core in range(NCORES):
        b, hg = core // HPC, core % HPC
        hs = slice(hg * HPC, (hg + 1) * HPC)

        def wmat(W, dt, scale):
            # fp8 weights pre-scaled by WSCALE to stay out of e4m3 denormals
            return np.ascontiguousarray(
                (W[hs].transpose(1, 0, 2).reshape(E, HPC * D) * scale).astype(dt)
            )

        def bpair(bias, scale):
            flat = (bias[hs].reshape(HPC * D) * scale).astype(np.float32)
            return np.ascontiguousarray(flat.reshape(HP, P).T)

        bv_flat = bv[hs].reshape(HPC * D).astype(np.float32)
        in_maps.append(
            {
                "xT": xTs[b],
                "xb": xbs[b],
                "wq": wmat(Wq, fp8, WSCALE),
                "wk": wmat(Wk, fp8, WSCALE),
                "wv": wmat(Wv, bf16, 1.0),
                "bq": bpair(bq, WSCALE),
                "bk": bpair(bk, WSCALE),
                "bv": np.ascontiguousarray(np.tile(bv_flat, (P, 1))),
                "wo": np.ascontiguousarray(
                    Wo[hg * HPC * D : (hg + 1) * HPC * D].astype(bf16)
                ),
                "mask": mask,
            }
        )
    return in_maps


def kernel(**inputs):
    global LAST_RESULTS
    x = np.asarray(inputs["x"], dtype=np.float32)
    Wq = np.asarray(inputs["Wq"], dtype=np.float32)
    bq = np.asarray(inputs["bq"], dtype=np.float32)
    Wk = np.asarray(inputs["Wk"], dtype=np.float32)
    bk = np.asarray(inputs["bk"], dtype=np.float32)
    Wv = np.asarray(inputs["Wv"], dtype=np.float32)
    bv = np.asarray(inputs["bv"], dtype=np.float32)
    Wo = np.asarray(inputs["Wo"], dtype=np.float32)
    bo = np.asarray(inputs["bo"], dtype=np.float32)

    nc = build()
    in_maps = make_in_maps(x, Wq, bq, Wk, bk, Wv, bv, Wo)
    res = run_bass_kernel_spmd(nc, in_maps, core_ids=list(range(NCORES)))
    LAST_RESULTS = res

    y = np.zeros((B, S, E), dtype=np.float32)
    for core in range(NCORES):
        y[core // HPC] += np.asarray(res.results[core]["y"]).astype(np.float32)
    y += bo[None, None, :]
    return y
